# revision 1
# baseline (speedup 1.0000x reference)
"""Distributed TRN2 Bass kernel for AdaptiveGraphTopology pairwise edge MLP.

reference:
    a = emb @ W1a.T ; b = emb @ W1b.T           (W1a, W1b = W1[:, :H], W1[:, H:])
    hidden = relu(a[:,None,:] + b[None,:,:] + b1)      # [N,N,H]
    scores = hidden . W2[0] + b2                       # [N,N]
    weights = sigmoid(scores), zeroed diag
    mask    = (weights > 0.5) & ~eye

Sharding: rows i split across 8 cores (128 rows each); everything else
replicated. No collectives: each core DMAs out its row block, host
concatenates.

Per-core compute:
    BT[h, j] = b_j[h]        (all j)    -- f32 matmul on device
    CT[h, i] = a_i[h]+b1[h]  (local i)  -- f32 matmul + bias on device
    loop over local i:
      X_i[h, j] = relu(BT[h, j] + CT[h, i])   (DVE/ACT, fp32r out)
      scores[i, :] += w2 . X_i  via fp32r matmul whose stationary is a
      sliding window over Z[128, 256] (w2 at column 128, zeros elsewhere):
      window [128-i : 256-i] places w2 in PE column i, so row i's scores
      land in PSUM partition i and the 128 iterations accumulate a full
      [128, 1024] score block (zero columns contribute exact zeros).
    then two more accumulating matmuls (stationary -1e30*I, moving a
    per-core one-hot eye_rows matrix) push the diagonal entries to -1e30,
    so the epilogue is just: weights = sigmoid(scores+b2)  (diag -> 0.0),
    mask = scores > -b2  (diag -> 0), with no separate diagonal masking.
"""
import numpy as np

N = 1024
H = 128
NCORES = 8
ROWS = N // NCORES  # 128 rows per core

_cache = {}


def _split_multiwaits(nc, limit=1):
    """This walrus build accepts only ONE semaphore wait/update per
    instruction; Tile emits several. Split extras onto adjacent NoOps."""
    import bass_rust

    f = nc.m.functions[0]
    engines = nc.engines

    def make_nop(engine_type):
        eng = engines[engine_type]
        inst = eng.nop(nofuse=True).ins
        for b in f.blocks:
            lst = b.instructions
            for k in range(len(lst) - 1, -1, -1):
                if lst[k] is inst:
                    lst.pop(k)
                    return inst
        return inst

    n_split = 0
    for b in f.blocks:
        insts = b.instructions
        i = 0
        while i < len(insts):
            inst = insts[i]
            si = inst.sync_info
            if si is None:
                i += 1
                continue
            waits = list(si.on_wait)
            ups = list(si.on_update)
            if len(waits) <= limit and len(ups) <= 1:
                i += 1
                continue
            pre = []
            post = []
            if len(waits) > limit:
                extra, waits = waits[: len(waits) - limit], waits[len(waits) - limit :]
                for w in extra:
                    nop = make_nop(inst.engine)
                    nop.sync_info = bass_rust.SyncInfo(on_wait=[w], on_update=[])
                    pre.append(nop)
            if len(ups) > 1:
                ups, extra_u = ups[:1], ups[1:]
                for u in extra_u:
                    nop = make_nop(inst.engine)
                    nop.sync_info = bass_rust.SyncInfo(on_wait=[], on_update=[u])
                    post.append(nop)
            inst.sync_info = bass_rust.SyncInfo(on_wait=waits, on_update=ups)
            insts[i:i] = pre
            i += len(pre)
            if post:
                insts[i + 1 : i + 1] = post
            n_split += 1
            i += 1
    return n_split


def _build(reps=1, loop_reps=1, mode="full"):
    import concourse.bass as bass
    import concourse.mybir as mybir
    from concourse.tile import TileContext

    nc = bass.Bass(trn_type="TRN2")
    f32 = mybir.dt.float32
    f32r = mybir.dt.float32r
    u8 = mybir.dt.uint8

    emb_t = nc.dram_tensor("emb_t", [H, N], f32, kind="ExternalInput")
    emb_rows_t = nc.dram_tensor("emb_rows_t", [H, ROWS], f32, kind="ExternalInput")
    w1a_t = nc.dram_tensor("w1a_t", [H, H], f32, kind="ExternalInput")
    w1b_t = nc.dram_tensor("w1b_t", [H, H], f32, kind="ExternalInput")
    b1_col = nc.dram_tensor("b1_col", [H, 1], f32, kind="ExternalInput")
    zbuf = nc.dram_tensor("zbuf", [H, 2 * H], f32, kind="ExternalInput")
    b2_col = nc.dram_tensor("b2_col", [H, 1], f32, kind="ExternalInput")
    negb2_col = nc.dram_tensor("negb2_col", [H, 1], f32, kind="ExternalInput")
    # rowcol[k] = global row index of local row k: used to build the one-hot
    # eye matrix on device (iota + is_equal) that injects -BIG into the
    # diagonal score entries via one accumulating matmul
    rowcol = nc.dram_tensor("rowcol", [ROWS, 1], f32, kind="ExternalInput")
    negbig_eye = nc.dram_tensor("negbig_eye", [H, H], f32, kind="ExternalInput")

    w_out = nc.dram_tensor("w_out", [ROWS, N], f32, kind="ExternalOutput")
    m_out = nc.dram_tensor("m_out", [ROWS, N], u8, kind="ExternalOutput")

    with TileContext(nc) as tc:
        with (
            tc.tile_pool(name="const", bufs=1) as cp,
            tc.tile_pool(name="xp", bufs=14) as xp,
            tc.tile_pool(name="pp", bufs=1, space="PSUM") as pp,
        ):
            emba_s = cp.tile([H, 512], f32, tag="emba")
            nc.sync.dma_start(out=emba_s[:], in_=emb_t[:, 0:512])
            embb_s = cp.tile([H, 512], f32, tag="embb")
            nc.sync.dma_start(out=embb_s[:], in_=emb_t[:, 512:1024])
            embr_s = cp.tile([H, ROWS], f32, tag="embr")
            nc.sync.dma_start(out=embr_s[:], in_=emb_rows_t[:])
            w1a_s = cp.tile([H, H], f32, tag="w1a")
            nc.sync.dma_start(out=w1a_s[:], in_=w1a_t[:])
            w1b_s = cp.tile([H, H], f32, tag="w1b")
            nc.sync.dma_start(out=w1b_s[:], in_=w1b_t[:])
            b1_s = cp.tile([H, 1], f32, tag="b1")
            nc.sync.dma_start(out=b1_s[:], in_=b1_col[:])
            z_s = cp.tile([H, 2 * H], f32, tag="z")
            nc.sync.dma_start(out=z_s[:], in_=zbuf[:])
            b2_s = cp.tile([H, 1], f32, tag="b2")
            nc.sync.dma_start(out=b2_s[:], in_=b2_col[:])
            nb2_s = cp.tile([H, 1], f32, tag="nb2")
            nc.sync.dma_start(out=nb2_s[:], in_=negb2_col[:])
            rc_s = cp.tile([ROWS, 1], f32, tag="rc")
            nc.sync.dma_start(out=rc_s[:], in_=rowcol[:])
            nbe_s = cp.tile([H, H], f32, tag="nbe")
            nc.sync.dma_start(out=nbe_s[:], in_=negbig_eye[:])

            # round f32r constants once
            zr_s = cp.tile([H, 2 * H], f32r, tag="zr")
            nc.vector.tensor_copy(zr_s[:], z_s[:])
            nber_s = cp.tile([H, H], f32r, tag="nber")
            nc.vector.tensor_copy(nber_s[:], nbe_s[:])

            # build the one-hot eye matrix on device: eyr[k, j] = (j == rowcol[k])
            it_s = cp.tile([ROWS, N], f32, tag="it")
            nc.gpsimd.iota(it_s[:], pattern=[[1, N]], base=0,
                           channel_multiplier=0,
                           allow_small_or_imprecise_dtypes=True)
            eyr_s = cp.tile([ROWS, N], f32r, tag="eyr")
            nc.vector.tensor_scalar(
                out=eyr_s[:],
                in0=it_s[:],
                scalar1=rc_s[:],
                scalar2=None,
                op0=mybir.AluOpType.is_equal,
            )

            # warm the PE HAM (clock gate) with dummy f32 matmuls while the
            # large input DMAs land, so prep + early main-loop matmuls run at
            # 2.4 GHz instead of the cold 1.2 GHz
            warm_ps = pp.tile([H, 128], f32, tag="warmp")
            for _w in range(12):
                nc.tensor.matmul(
                    warm_ps[:], w1a_s[:], w1a_s[:], start=True, stop=True
                )

            # force the sigmoid ACT table set to load during prep, so the
            # epilogue sigmoid doesn't pay a ~2.7us mid-kernel table swap
            # (relu/identity are filler entries in every set); reading
            # warm_ps also keeps the warm matmuls alive through DCE
            warm_s = cp.tile([H, 1], f32, tag="warm")
            nc.scalar.activation(
                warm_s[:], warm_ps[:, 0:1], mybir.ActivationFunctionType.Sigmoid
            )

            if loop_reps > 1:
                with tc.For_i(0, loop_reps, 1):
                    _body_once(nc, tc, cp, xp, pp, mybir, f32, f32r, u8,
                               (emba_s, embb_s), embr_s, w1a_s, w1b_s, b1_s,
                               zr_s, b2_s, nb2_s, eyr_s, nber_s, w_out, m_out,
                               mode)
            else:
                for _rep in range(reps):
                    _body_once(nc, tc, cp, xp, pp, mybir, f32, f32r, u8,
                               (emba_s, embb_s), embr_s, w1a_s, w1b_s, b1_s,
                               zr_s, b2_s, nb2_s, eyr_s, nber_s, w_out, m_out,
                               mode)

    _split_multiwaits(nc)
    return nc


def _body_once(nc, tc, cp, xp, pp, mybir, f32, f32r, u8,
               embt_halves, embr_s, w1a_s, w1b_s, b1_s, zr_s, b2_s, nb2_s,
               eyr_s, nber_s, w_out, m_out, mode="full"):
    emba_s, embb_s = embt_halves
    if mode == "empty":
        return
    if True:
        if True:
            # BT = W1b @ embT  (f32, exact): psum half per matmul; each half
            # depends only on its own emb DMA, and the PSUM->SBUF copies run
            # on different engines so they overlap
            bt_ps = pp.tile([H, N], f32, tag="btp")
            nc.tensor.matmul(
                bt_ps[:, 0:512], w1b_s[:], emba_s[:], start=True, stop=True
            )
            nc.tensor.matmul(
                bt_ps[:, 512:1024], w1b_s[:], embb_s[:], start=True, stop=True
            )
            bt_s = cp.tile([H, N], f32, tag="bt")
            nc.vector.tensor_copy(bt_s[:, 0:512], bt_ps[:, 0:512])
            nc.scalar.copy(bt_s[:, 512:1024], bt_ps[:, 512:1024])

            # CT = W1a @ embT_rows + b1  (f32, exact)
            ct_ps = pp.tile([H, ROWS], f32, tag="ctp")
            nc.tensor.matmul(ct_ps[:], w1a_s[:], embr_s[:], start=True, stop=True)
            ct_s = cp.tile([H, ROWS], f32, tag="ct")
            nc.scalar.activation(
                ct_s[:], ct_ps[:], mybir.ActivationFunctionType.Identity, bias=b1_s[:]
            )

            # main loop: accumulate scores into PSUM [128 rows, 1024 cols]
            sc_ps = pp.tile([ROWS, N], f32, tag="scores")
            # initialize each scores bank with -BIG at the diagonal entries
            # (zeros elsewhere): out[k, j] = -BIG*eye[k, j]; keeps the
            # epilogue off the critical tail
            for h0 in (0, 512):
                nc.tensor.matmul(
                    sc_ps[:, h0 : h0 + 512],
                    nber_s[:],
                    eyr_s[:, h0 : h0 + 512],
                    start=True,
                    stop=False,
                )
            xfix = None
            if mode == "nogen":
                xfix = cp.tile([H, N], f32r, tag="xfix")
                nc.vector.tensor_copy(xfix[:, 0:256], zr_s[:])
            if mode == "full2":
                # col-group tiled reduction: 32-wide stationaries, 4 strips
                for k in range(32):
                    for g in range(4):
                        i = 32 * g + k
                        x = xp.tile([H, N], f32r, tag="x")
                        if (i * 5) % 13 < 5:
                            nc.scalar.activation(
                                x[:],
                                bt_s[:],
                                mybir.ActivationFunctionType.Relu,
                                bias=ct_s[:, i : i + 1],
                            )
                        else:
                            nc.vector.tensor_scalar(
                                out=x[:],
                                in0=bt_s[:],
                                scalar1=ct_s[:, i : i + 1],
                                scalar2=0.0,
                                op0=mybir.AluOpType.add,
                                op1=mybir.AluOpType.max,
                            )
                        lhsT = zr_s[:, H - k : H + 32 - k]
                        for h0 in (0, 512):
                            nc.tensor.matmul(
                                sc_ps[32 * g : 32 * g + 32, h0 : h0 + 512],
                                lhsT,
                                x[:, h0 : h0 + 512],
                                start=(k == 0),
                                stop=(k == 31),
                                tile_position=(0, 32 * g),
                            )
                _epilogue(nc, cp, mybir, f32, u8, sc_ps, b2_s, nb2_s, w_out, m_out)
                return

            for i in range(ROWS):
                if mode != "nogen":
                    x = xp.tile([H, N], f32r, tag="x")
                    if mode == "actgen" or (mode != "dvegen" and i % 3 == 1):
                        # ACT path: relu(in + bias), ~1147ns
                        nc.scalar.activation(
                            x[:],
                            bt_s[:],
                            mybir.ActivationFunctionType.Relu,
                            bias=ct_s[:, i : i + 1],
                        )
                    else:
                        # DVE path: (in + c_i) then max(.,0), ~720ns
                        nc.vector.tensor_scalar(
                            out=x[:],
                            in0=bt_s[:],
                            scalar1=ct_s[:, i : i + 1],
                            scalar2=0.0,
                            op0=mybir.AluOpType.add,
                            op1=mybir.AluOpType.max,
                        )
                else:
                    x = xfix
                if mode == "nomm":
                    continue
                lhsT = zr_s[:, H - i : 2 * H - i]
                nc.tensor.matmul(
                    sc_ps[:, 0:512],
                    lhsT,
                    x[:, 0:512],
                    start=False,
                    stop=(i == ROWS - 1),
                )
                nc.tensor.matmul(
                    sc_ps[:, 512:1024],
                    lhsT,
                    x[:, 512:1024],
                    start=False,
                    stop=(i == ROWS - 1),
                )
            if mode == "nomm":
                return

            _epilogue(nc, cp, mybir, f32, u8, sc_ps, b2_s, nb2_s, w_out, m_out)


def _epilogue(nc, cp, mybir, f32, u8, sc_ps, b2_s, nb2_s, w_out, m_out):
    # diagonal score entries hold -BIG: sigmoid -> 0 weight, is_gt -> 0 mask
    sig_s = cp.tile([ROWS, N], f32, tag="sig")
    nc.scalar.activation(
        sig_s[:], sc_ps[:], mybir.ActivationFunctionType.Sigmoid, bias=b2_s[:]
    )
    nc.sync.dma_start(out=w_out[:], in_=sig_s[:])

    m_s = cp.tile([ROWS, N], u8, tag="m")
    nc.vector.tensor_scalar(
        out=m_s[:],
        in0=sc_ps[:],
        scalar1=nb2_s[:],
        scalar2=None,
        op0=mybir.AluOpType.is_gt,
    )
    nc.sync.dma_start(out=m_out[:], in_=m_s[:])


def _build_in_maps(inputs):
    node_emb = np.asarray(inputs["node_emb"], dtype=np.float32)
    W1 = np.asarray(inputs["W1"], dtype=np.float32)
    b1 = np.asarray(inputs["b1"], dtype=np.float32)
    W2 = np.asarray(inputs["W2"], dtype=np.float32)
    b2 = np.asarray(inputs["b2"], dtype=np.float32)

    emb_t = np.ascontiguousarray(node_emb.T)  # [H, N]
    w1a_t = np.ascontiguousarray(W1[:, :H].T)  # [e, h]
    w1b_t = np.ascontiguousarray(W1[:, H:].T)
    b1_col = np.ascontiguousarray(b1.reshape(H, 1))
    zbuf = np.zeros((H, 2 * H), dtype=np.float32)
    zbuf[:, H] = W2[0]
    b2v = np.float32(b2.reshape(-1)[0])
    b2_col = np.full((H, 1), b2v, dtype=np.float32)
    negb2_col = -b2_col

    negbig_eye = np.zeros((H, H), dtype=np.float32)
    np.fill_diagonal(negbig_eye, np.float32(-1e30))

    in_maps = []
    for c in range(NCORES):
        r0 = c * ROWS
        in_maps.append(
            {
                "emb_t": emb_t,
                "emb_rows_t": np.ascontiguousarray(emb_t[:, r0 : r0 + ROWS]),
                "w1a_t": w1a_t,
                "w1b_t": w1b_t,
                "b1_col": b1_col,
                "zbuf": zbuf,
                "b2_col": b2_col,
                "negb2_col": negb2_col,
                "rowcol": (r0 + np.arange(ROWS, dtype=np.float32)).reshape(ROWS, 1),
                "negbig_eye": negbig_eye,
            }
        )
    return in_maps


def _make_runner(nc):
    """Build a reusable jitted runner (mirrors bass2jax.run_bass_via_pjrt,
    but cached so repeated kernel() calls skip re-tracing/compiling)."""
    import jax
    import concourse.mybir as mybir
    from jax.sharding import Mesh, PartitionSpec

    try:
        from jax.experimental.shard_map import shard_map
    except ImportError:
        from jax.shard_map import shard_map

    from concourse.bass2jax import (
        _bass_exec_p,
        install_neuronx_cc_hook,
        partition_id_tensor,
    )

    install_neuronx_cc_hook()
    partition_name = nc.partition_id_tensor.name if nc.partition_id_tensor else None

    in_names, out_names, out_avals, zero_outs = [], [], [], []
    for alloc in nc.m.functions[0].allocations:
        if not isinstance(alloc, mybir.MemoryLocationSet):
            continue
        name = alloc.memorylocations[0].name
        if alloc.kind == "ExternalInput":
            if name != partition_name:
                in_names.append(name)
        elif alloc.kind == "ExternalOutput":
            out_names.append(name)
            shape = tuple(alloc.tensor_shape)
            dtype = mybir.dt.np(alloc.dtype)
            out_avals.append(jax.core.ShapedArray(shape, dtype))
            zero_outs.append(np.zeros(shape, dtype))
    n_params = len(in_names)
    all_in_names = list(in_names) + list(out_names)
    if partition_name is not None:
        all_in_names.append(partition_name)

    def _body(*args):
        operands = list(args)
        if partition_name is not None:
            operands.append(partition_id_tensor())
        return tuple(
            _bass_exec_p.bind(
                *operands,
                out_avals=tuple(out_avals),
                in_names=tuple(all_in_names),
                out_names=tuple(out_names),
                lowering_input_output_aliases=(),
                sim_require_finite=True,
                sim_require_nnan=True,
                nc=nc,
            )
        )

    devices = jax.devices()[:NCORES]
    mesh = Mesh(np.asarray(devices), ("core",))
    n_outs = len(out_avals)
    # only these inputs differ per core; the rest are replicated and ship
    # to the devices once instead of 8 concatenated copies
    per_core_names = {"emb_rows_t", "rowcol"}
    in_specs = tuple(
        PartitionSpec("core") if n in per_core_names else PartitionSpec(None)
        for n in in_names
    ) + (PartitionSpec("core"),) * n_outs
    out_specs = (PartitionSpec("core"),) * n_outs
    fn = jax.jit(
        shard_map(_body, mesh=mesh, in_specs=in_specs, out_specs=out_specs,
                  check_rep=False),
        keep_unused=True,
    )
    concat_zeros = [
        np.zeros((NCORES * z.shape[0], *z.shape[1:]), z.dtype) for z in zero_outs
    ]
    return fn, in_names, out_names, out_avals, concat_zeros, per_core_names


def _run_cached(in_maps):
    import jax

    if "runner" not in _cache:
        _cache["runner"] = _make_runner(_cache["nc"])
    fn, in_names, out_names, out_avals, concat_zeros, per_core_names = _cache["runner"]
    concat_in = [
        np.concatenate([np.asarray(m[name]) for m in in_maps], axis=0)
        if name in per_core_names
        else np.asarray(in_maps[0][name])
        for name in in_names
    ]
    out_arrs = fn(*concat_in, *concat_zeros)
    jax.block_until_ready(out_arrs)
    res = {}
    for i, name in enumerate(out_names):
        res[name] = np.asarray(out_arrs[i]).reshape(
            NCORES, *out_avals[i].shape
        )
    return res


def kernel(node_emb, W1, b1, W2, b2, temperature=None, **_ignored):
    import time

    if "nc" not in _cache:
        _cache["nc"] = _build()

    in_maps = _build_in_maps(
        {"node_emb": node_emb, "W1": W1, "b1": b1, "W2": W2, "b2": b2}
    )
    # the device occasionally reports NRT_EXEC_UNIT_UNRECOVERABLE if a prior
    # process wedged it; it self-recovers after ~30s, so retry those (and only
    # those) with backoff
    for attempt in range(3):
        try:
            res = _run_cached(in_maps)
            break
        except Exception as e:  # noqa: BLE001
            msg = str(e)
            transient = (
                "UNRECOVERABLE" in msg
                or "unrecoverable" in msg
                or "UNAVAILABLE" in msg
            )
            if attempt == 2 or not transient:
                raise
            time.sleep(30 * (attempt + 1))
    weights = np.concatenate([res["w_out"][c] for c in range(NCORES)], axis=0)
    mask = np.concatenate([res["m_out"][c] for c in range(NCORES)], axis=0).astype(bool)
    return weights, mask



# revision 10
# speedup vs baseline: 2.8750x; 2.8750x over previous
"""Distributed TRN2 Bass kernel for AdaptiveGraphTopology pairwise edge MLP.

reference:
    a = emb @ W1a.T ; b = emb @ W1b.T           (W1a, W1b = W1[:, :H], W1[:, H:])
    hidden = relu(a[:,None,:] + b[None,:,:] + b1)      # [N,N,H]
    scores = hidden . W2[0] + b2                       # [N,N]
    weights = sigmoid(scores), zeroed diag
    mask    = (weights > 0.5) & ~eye

Sharding: rows i split across 8 cores (128 rows each); everything else
replicated. No collectives: each core DMAs out its row block, host
concatenates.

Per-core compute (v2):
    BT[h, j] = b_j[h]        (all j)    -- f32 matmul, exact, kept in PSUM
    CT[h, i] = a_i[h]+b1[h]  (local i)  -- f32 matmul + bias
    bt16     = fp16 round of BT in SBUF (DVE/ACT copies)
    loop over 32 slots k, each covering rows i = 32g+k for col-groups g=0..3:
      X_i[h, j] = relu(BT[h, j] + CT[h, i])
        - most rows: DVE tensor_scalar fp16->fp16 (4x perf mode, ~450ns)
        - some rows: ACT activation from f32 PSUM -> f32r (exact, ~1.2us)
      scores[i, :] += w2 . X_i  via col-group-tiled matmuls: each group g
      uses a 32-wide sliding window over Z[128, 256] (w2 at column 128,
      zeros elsewhere) as stationary at tile_position (0, 32g); window
      [128-k : 160-k] places w2 in PE column k of the group, so row
      (32g+k)'s scores land in PSUM partition 32g+k. The four groups'
      matmuls execute concurrently on disjoint 32-column strips of the
      PE array (~3-4x faster than full-width serial matmuls).
    diagonal entries are pushed to -1e30 by per-group accumulating matmuls
    (stationary -1e30*I columns, moving a one-hot eye_rows matrix), so the
    epilogue is just: weights = sigmoid(scores+b2) (diag -> 0.0),
    mask = weights > 0.5 (diag -> 0), with no separate diagonal masking.
"""
import numpy as np

N = 1024
H = 128
NCORES = 8
ROWS = N // NCORES  # 128 rows per core
NG = 4              # PE col-groups
GW = ROWS // NG     # 32 rows per group
# rows generated on ACT (exact f32 path); the rest on DVE (fp16 4x path).
# chosen to balance engine busy time: ACT ~1.15us/row vs DVE ~0.45us/row
ACT_ROWS = frozenset(
    {3 * GW + k for k in range(GW)} | {2 * GW + k for k in (10, 21, 31)}
)

_cache = {}


def _split_multiwaits(nc, limit=1):
    """This walrus build accepts only ONE semaphore wait/update per
    instruction; Tile emits several. Split extras onto adjacent NoOps."""
    import bass_rust

    f = nc.m.functions[0]
    engines = nc.engines

    def make_nop(engine_type):
        eng = engines[engine_type]
        inst = eng.nop(nofuse=True).ins
        for b in f.blocks:
            lst = b.instructions
            for k in range(len(lst) - 1, -1, -1):
                if lst[k] is inst:
                    lst.pop(k)
                    return inst
        return inst

    n_split = 0
    for b in f.blocks:
        insts = b.instructions
        i = 0
        while i < len(insts):
            inst = insts[i]
            si = inst.sync_info
            if si is None:
                i += 1
                continue
            waits = list(si.on_wait)
            ups = list(si.on_update)
            if len(waits) <= limit and len(ups) <= 1:
                i += 1
                continue
            pre = []
            post = []
            if len(waits) > limit:
                extra, waits = waits[: len(waits) - limit], waits[len(waits) - limit :]
                for w in extra:
                    nop = make_nop(inst.engine)
                    nop.sync_info = bass_rust.SyncInfo(on_wait=[w], on_update=[])
                    pre.append(nop)
            if len(ups) > 1:
                ups, extra_u = ups[:1], ups[1:]
                for u in extra_u:
                    nop = make_nop(inst.engine)
                    nop.sync_info = bass_rust.SyncInfo(on_wait=[], on_update=[u])
                    post.append(nop)
            inst.sync_info = bass_rust.SyncInfo(on_wait=waits, on_update=ups)
            insts[i:i] = pre
            i += len(pre)
            if post:
                insts[i + 1 : i + 1] = post
            n_split += 1
            i += 1
    return n_split


def _build(reps=1, loop_reps=1, mode="full"):
    import concourse.bass as bass
    import concourse.mybir as mybir
    from concourse.tile import TileContext

    nc = bass.Bass(trn_type="TRN2")
    f32 = mybir.dt.float32
    f32r = mybir.dt.float32r
    f16 = mybir.dt.float16
    u8 = mybir.dt.uint8

    emb_t = nc.dram_tensor("emb_t", [H, N], f32, kind="ExternalInput")
    emb_rows_t = nc.dram_tensor("emb_rows_t", [H, ROWS], f32, kind="ExternalInput")
    w1a_t = nc.dram_tensor("w1a_t", [H, H], f32, kind="ExternalInput")
    w1b_t = nc.dram_tensor("w1b_t", [H, H], f32, kind="ExternalInput")
    b1_col = nc.dram_tensor("b1_col", [H, 1], f32, kind="ExternalInput")
    zbuf = nc.dram_tensor("zbuf", [H, 2 * H], f32, kind="ExternalInput")
    b2_col = nc.dram_tensor("b2_col", [H, 1], f32, kind="ExternalInput")
    # rowcol[k] = global row index of local row k: used to build the one-hot
    # eye matrix on device (iota + is_equal) that injects -BIG into the
    # diagonal score entries via accumulating matmuls
    rowcol = nc.dram_tensor("rowcol", [ROWS, 1], f32, kind="ExternalInput")
    negbig_eye = nc.dram_tensor("negbig_eye", [H, H], f32, kind="ExternalInput")

    w_out = nc.dram_tensor("w_out", [ROWS, N], f32, kind="ExternalOutput")
    m_out = nc.dram_tensor("m_out", [ROWS, N], u8, kind="ExternalOutput")

    with TileContext(nc) as tc:
        with (
            tc.tile_pool(name="const", bufs=1) as cp,
            tc.tile_pool(name="bt", bufs=2) as btp,
            tc.tile_pool(name="x16", bufs=12) as xp16,
            tc.tile_pool(name="x32", bufs=5) as xp32,
            tc.tile_pool(name="ppb", bufs=2, space="PSUM") as ppb,  # btp only: 2x2 banks
            tc.tile_pool(name="pps", bufs=1, space="PSUM") as pps,  # warm+ct+scores: 4 banks
        ):
            emba_s = cp.tile([H, 512], f32, tag="emba")
            nc.sync.dma_start(out=emba_s[:], in_=emb_t[:, 0:512])
            embb_s = cp.tile([H, 512], f32, tag="embb")
            nc.sync.dma_start(out=embb_s[:], in_=emb_t[:, 512:1024])
            embr_s = cp.tile([H, ROWS], f32, tag="embr")
            nc.sync.dma_start(out=embr_s[:], in_=emb_rows_t[:])
            w1a_s = cp.tile([H, H], f32, tag="w1a")
            nc.sync.dma_start(out=w1a_s[:], in_=w1a_t[:])
            w1b_s = cp.tile([H, H], f32, tag="w1b")
            nc.sync.dma_start(out=w1b_s[:], in_=w1b_t[:])
            b1_s = cp.tile([H, 1], f32, tag="b1")
            nc.sync.dma_start(out=b1_s[:], in_=b1_col[:])
            z_s = cp.tile([H, 2 * H], f32, tag="z")
            nc.sync.dma_start(out=z_s[:], in_=zbuf[:])
            b2_s = cp.tile([H, 1], f32, tag="b2")
            nc.sync.dma_start(out=b2_s[:], in_=b2_col[:])
            rc_s = cp.tile([ROWS, 1], f32, tag="rc")
            nc.sync.dma_start(out=rc_s[:], in_=rowcol[:])
            nbe_s = cp.tile([H, H], f32, tag="nbe")
            nc.sync.dma_start(out=nbe_s[:], in_=negbig_eye[:])

            # fp16 copies: walrus requires matmul stationary/moving dtypes to
            # match, and f32/f32r matmuls cannot use col-group tile_position
            # (s3d3_mm_valid_dst_partition) -- so everything the col-group
            # matmuls touch is fp16. The -BIG diag value is -32768 (exact in
            # fp16; saturates sigmoid to 0 through the f32 PSUM accumulate).
            z16_s = cp.tile([H, 2 * H], f16, tag="z16")
            nc.vector.tensor_copy(z16_s[:], z_s[:])
            nber_s = cp.tile([H, H], f16, tag="nber")
            nc.vector.tensor_copy(nber_s[:], nbe_s[:])

            # build the one-hot eye matrix on device: eyr[k, j] = (j == rowcol[k])
            it_s = cp.tile([ROWS, N], f32, tag="it")
            nc.gpsimd.iota(it_s[:], pattern=[[1, N]], base=0,
                           channel_multiplier=0,
                           allow_small_or_imprecise_dtypes=True)
            eyr_s = cp.tile([ROWS, N], f16, tag="eyr")
            nc.vector.tensor_scalar(
                out=eyr_s[:],
                in0=it_s[:],
                scalar1=rc_s[:],
                scalar2=None,
                op0=mybir.AluOpType.is_equal,
            )

            # warm the PE HAM (clock gate) with dummy f32 matmuls while the
            # large input DMAs land, so prep + early main-loop matmuls run at
            # 2.4 GHz instead of the cold 1.2 GHz
            warm_ps = pps.tile([H, 128], f32, tag="warmp")
            for _w in range(12):
                nc.tensor.matmul(
                    warm_ps[:], w1a_s[:], w1a_s[:], start=True, stop=True
                )

            # force the sigmoid ACT table set to load during prep, so the
            # epilogue sigmoid doesn't pay a ~2.7us mid-kernel table swap
            # (relu/identity are filler entries in every set); reading
            # warm_ps also keeps the warm matmuls alive through DCE
            warm_s = cp.tile([H, 1], f32, tag="warm")
            nc.scalar.activation(
                warm_s[:], warm_ps[:, 0:1], mybir.ActivationFunctionType.Sigmoid
            )

            args = (nc, tc, cp, btp, xp16, xp32, ppb, pps, mybir,
                    f32, f32r, f16, u8,
                    (emba_s, embb_s), embr_s, w1a_s, w1b_s, b1_s,
                    z16_s, b2_s, eyr_s, nber_s, w_out, m_out, mode)
            if loop_reps > 1:
                with tc.For_i(0, loop_reps, 1):
                    _body_once(*args)
            else:
                for _rep in range(reps):
                    _body_once(*args)

    _split_multiwaits(nc)
    return nc


def _body_once(nc, tc, cp, btp, xp16, xp32, ppb, pps, mybir,
               f32, f32r, f16, u8,
               embt_halves, embr_s, w1a_s, w1b_s, b1_s, z16_s, b2_s,
               eyr_s, nber_s, w_out, m_out, mode="full"):
    emba_s, embb_s = embt_halves
    if mode == "empty":
        return

    # BT = W1b @ embT  (f32, exact): stays in PSUM for the ACT gen path;
    # fp16-rounded copy in SBUF for the DVE gen path
    bt_ps = ppb.tile([H, N], f32, tag="btp")
    nc.tensor.matmul(
        bt_ps[:, 0:512], w1b_s[:], emba_s[:], start=True, stop=True
    )
    nc.tensor.matmul(
        bt_ps[:, 512:1024], w1b_s[:], embb_s[:], start=True, stop=True
    )
    bt16_s = btp.tile([H, N], f16, tag="bt16")
    nc.vector.tensor_copy(bt16_s[:, 0:512], bt_ps[:, 0:512])
    nc.scalar.copy(bt16_s[:, 512:1024], bt_ps[:, 512:1024])

    # CT = W1a @ embT_rows + b1  (f32, exact)
    ct_ps = pps.tile([H, ROWS], f32, tag="ctp")
    nc.tensor.matmul(ct_ps[:], w1a_s[:], embr_s[:], start=True, stop=True)
    ct_s = btp.tile([H, ROWS], f32, tag="ct")
    nc.scalar.activation(
        ct_s[:], ct_ps[:], mybir.ActivationFunctionType.Identity, bias=b1_s[:]
    )

    # scores PSUM [128 rows, 1024 cols]; initialize each col-group slice
    # with -BIG at the diagonal entries (zeros elsewhere) so the epilogue
    # needs no separate diagonal masking
    sc_ps = pps.tile([ROWS, N], f32, tag="scores")
    for h0 in (0, 512):
        for g in range(NG):
            nc.tensor.matmul(
                sc_ps[GW * g : GW * (g + 1), h0 : h0 + 512],
                nber_s[:, GW * g : GW * (g + 1)],
                eyr_s[:, h0 : h0 + 512],
                start=True,
                stop=False,
                tile_position=(0, GW * g),
            )

    if mode == "wide":
        # ablation: old full-width sliding-window matmuls (no col groups)
        for i in range(ROWS):
            x = xp16.tile([H, N], f16, tag="x")
            nc.vector.tensor_scalar(
                out=x[:], in0=bt16_s[:], scalar1=ct_s[:, i : i + 1],
                scalar2=0.0, op0=mybir.AluOpType.add, op1=mybir.AluOpType.max,
            )
            lhsT = z16_s[:, H - i : 2 * H - i]
            for h0 in (0, 512):
                nc.tensor.matmul(
                    sc_ps[:, h0 : h0 + 512], lhsT, x[:, h0 : h0 + 512],
                    start=False, stop=(i == ROWS - 1),
                )
        _epilogue(nc, cp, mybir, f32, u8, sc_ps, b2_s, w_out, m_out)
        return

    # main loop: 32 slots, each generating + reducing rows {32g+k} for the
    # four col-groups concurrently
    for k in range(GW):
        xs = []
        for g in range(NG):
            i = GW * g + k
            if mode != "nogen" and i in ACT_ROWS:
                # ACT path: exact f32 read from PSUM, fp16 out
                x = xp16.tile([H, N], f16, tag="xa")
                nc.scalar.activation(
                    x[:],
                    bt_ps[:],
                    mybir.ActivationFunctionType.Relu,
                    bias=ct_s[:, i : i + 1],
                )
            elif mode != "nogen" or (k == 0 and g == 0):
                # DVE path: fp16 in/out hits the 4x perf mode
                x = xp16.tile([H, N], f16, tag="x")
                nc.vector.tensor_scalar(
                    out=x[:],
                    in0=bt16_s[:],
                    scalar1=ct_s[:, i : i + 1],
                    scalar2=0.0,
                    op0=mybir.AluOpType.add,
                    op1=mybir.AluOpType.max,
                )
                if mode == "nogen":
                    xfix = x
            else:
                x = xfix
            xs.append(x)
        if mode == "nomm":
            continue
        # w2 sits at column H of the window buffer; window [H-k : H+32-k]
        # places it in PE column k of each 32-wide col-group, so row
        # 32g+k's scores land in PSUM partition 32g+k. Interleave groups
        # (g-major) so the four matmuls run concurrently on disjoint
        # col-group strips.
        for h0 in (0, 512):
            for g in range(NG):
                nc.tensor.matmul(
                    sc_ps[GW * g : GW * (g + 1), h0 : h0 + 512],
                    z16_s[:, H - k : H + GW - k],
                    xs[g][:, h0 : h0 + 512],
                    start=False,
                    stop=(k == GW - 1),
                    tile_position=(0, GW * g),
                )

    if mode == "nomm":
        return
    _epilogue(nc, cp, mybir, f32, u8, sc_ps, b2_s, w_out, m_out)


def _epilogue(nc, cp, mybir, f32, u8, sc_ps, b2_s, w_out, m_out):
    # diagonal score entries hold -BIG: sigmoid -> 0 weight, is_gt -> 0 mask
    sig_s = cp.tile([ROWS, N], f32, tag="sig")
    nc.scalar.activation(
        sig_s[:], sc_ps[:], mybir.ActivationFunctionType.Sigmoid, bias=b2_s[:]
    )
    nc.sync.dma_start(out=w_out[:], in_=sig_s[:])

    m_s = cp.tile([ROWS, N], u8, tag="m")
    nc.vector.tensor_scalar(
        out=m_s[:],
        in0=sig_s[:],
        scalar1=0.5,
        scalar2=None,
        op0=mybir.AluOpType.is_gt,
    )
    nc.sync.dma_start(out=m_out[:], in_=m_s[:])


def _build_in_maps(inputs):
    node_emb = np.asarray(inputs["node_emb"], dtype=np.float32)
    W1 = np.asarray(inputs["W1"], dtype=np.float32)
    b1 = np.asarray(inputs["b1"], dtype=np.float32)
    W2 = np.asarray(inputs["W2"], dtype=np.float32)
    b2 = np.asarray(inputs["b2"], dtype=np.float32)

    emb_t = np.ascontiguousarray(node_emb.T)  # [H, N]
    w1a_t = np.ascontiguousarray(W1[:, :H].T)  # [e, h]
    w1b_t = np.ascontiguousarray(W1[:, H:].T)
    b1_col = np.ascontiguousarray(b1.reshape(H, 1))
    zbuf = np.zeros((H, 2 * H), dtype=np.float32)
    zbuf[:, H] = W2[0]
    b2v = np.float32(b2.reshape(-1)[0])
    b2_col = np.full((H, 1), b2v, dtype=np.float32)

    negbig_eye = np.zeros((H, H), dtype=np.float32)
    np.fill_diagonal(negbig_eye, np.float32(-32768.0))

    in_maps = []
    for c in range(NCORES):
        r0 = c * ROWS
        in_maps.append(
            {
                "emb_t": emb_t,
                "emb_rows_t": np.ascontiguousarray(emb_t[:, r0 : r0 + ROWS]),
                "w1a_t": w1a_t,
                "w1b_t": w1b_t,
                "b1_col": b1_col,
                "zbuf": zbuf,
                "b2_col": b2_col,
                "rowcol": (r0 + np.arange(ROWS, dtype=np.float32)).reshape(ROWS, 1),
                "negbig_eye": negbig_eye,
            }
        )
    return in_maps


def _make_runner(nc):
    """Build a reusable jitted runner (mirrors bass2jax.run_bass_via_pjrt,
    but cached so repeated kernel() calls skip re-tracing/compiling)."""
    import jax
    import concourse.mybir as mybir
    from jax.sharding import Mesh, PartitionSpec

    try:
        from jax.experimental.shard_map import shard_map
    except ImportError:
        from jax.shard_map import shard_map

    from concourse.bass2jax import (
        _bass_exec_p,
        install_neuronx_cc_hook,
        partition_id_tensor,
    )

    install_neuronx_cc_hook()
    partition_name = nc.partition_id_tensor.name if nc.partition_id_tensor else None

    in_names, out_names, out_avals, zero_outs = [], [], [], []
    for alloc in nc.m.functions[0].allocations:
        if not isinstance(alloc, mybir.MemoryLocationSet):
            continue
        name = alloc.memorylocations[0].name
        if alloc.kind == "ExternalInput":
            if name != partition_name:
                in_names.append(name)
        elif alloc.kind == "ExternalOutput":
            out_names.append(name)
            shape = tuple(alloc.tensor_shape)
            dtype = mybir.dt.np(alloc.dtype)
            out_avals.append(jax.core.ShapedArray(shape, dtype))
            zero_outs.append(np.zeros(shape, dtype))
    n_params = len(in_names)
    all_in_names = list(in_names) + list(out_names)
    if partition_name is not None:
        all_in_names.append(partition_name)

    def _body(*args):
        operands = list(args)
        if partition_name is not None:
            operands.append(partition_id_tensor())
        return tuple(
            _bass_exec_p.bind(
                *operands,
                out_avals=tuple(out_avals),
                in_names=tuple(all_in_names),
                out_names=tuple(out_names),
                lowering_input_output_aliases=(),
                sim_require_finite=True,
                sim_require_nnan=True,
                nc=nc,
            )
        )

    devices = jax.devices()[:NCORES]
    mesh = Mesh(np.asarray(devices), ("core",))
    n_outs = len(out_avals)
    # only these inputs differ per core; the rest are replicated and ship
    # to the devices once instead of 8 concatenated copies
    per_core_names = {"emb_rows_t", "rowcol"}
    in_specs = tuple(
        PartitionSpec("core") if n in per_core_names else PartitionSpec(None)
        for n in in_names
    ) + (PartitionSpec("core"),) * n_outs
    out_specs = (PartitionSpec("core"),) * n_outs
    fn = jax.jit(
        shard_map(_body, mesh=mesh, in_specs=in_specs, out_specs=out_specs,
                  check_rep=False),
        keep_unused=True,
    )
    concat_zeros = [
        np.zeros((NCORES * z.shape[0], *z.shape[1:]), z.dtype) for z in zero_outs
    ]
    return fn, in_names, out_names, out_avals, concat_zeros, per_core_names


def _run_cached(in_maps):
    import jax

    if "runner" not in _cache:
        _cache["runner"] = _make_runner(_cache["nc"])
    fn, in_names, out_names, out_avals, concat_zeros, per_core_names = _cache["runner"]
    concat_in = [
        np.concatenate([np.asarray(m[name]) for m in in_maps], axis=0)
        if name in per_core_names
        else np.asarray(in_maps[0][name])
        for name in in_names
    ]
    out_arrs = fn(*concat_in, *concat_zeros)
    jax.block_until_ready(out_arrs)
    res = {}
    for i, name in enumerate(out_names):
        res[name] = np.asarray(out_arrs[i]).reshape(
            NCORES, *out_avals[i].shape
        )
    return res


def kernel(node_emb, W1, b1, W2, b2, temperature=None, **_ignored):
    import time

    if "nc" not in _cache:
        _cache["nc"] = _build()

    in_maps = _build_in_maps(
        {"node_emb": node_emb, "W1": W1, "b1": b1, "W2": W2, "b2": b2}
    )
    # the device occasionally reports NRT_EXEC_UNIT_UNRECOVERABLE if a prior
    # process wedged it; it self-recovers after ~30s, so retry those (and only
    # those) with backoff
    for attempt in range(3):
        try:
            res = _run_cached(in_maps)
            break
        except Exception as e:  # noqa: BLE001
            msg = str(e)
            transient = (
                "UNRECOVERABLE" in msg
                or "unrecoverable" in msg
                or "UNAVAILABLE" in msg
            )
            if attempt == 2 or not transient:
                raise
            time.sleep(30 * (attempt + 1))
    weights = np.concatenate([res["w_out"][c] for c in range(NCORES)], axis=0)
    mask = np.concatenate([res["m_out"][c] for c in range(NCORES)], axis=0).astype(bool)
    return weights, mask
